# revision 15
# baseline (speedup 1.0000x reference)
"""Trainium2 Bass kernel for the LSTM-unit problem (B=262144, I=H=C=O=128).

Strategy (data-parallel over 8 NeuronCores, batch-sharded):
  - Host pre-transposes the big activations to [feature, batch] layout so the
    on-chip pipeline needs zero transposes: the TensorE contraction dim
    (partitions) is the feature dim for both GEMMs, gate biases become
    per-partition (fused into ScalarE activations for free), and the second
    GEMM consumes the h tiles the first stage produced.
  - GEMM operands use float32r (reduced-precision fp32 matmul mode): full
    1 elem/cycle PE rate at N=512 with ~1.6e-4 relative error.
  - Per core: stream 16 supertiles of 2048 batch rows; each supertile is
    4 waves of 512 batch columns:
      gates.T[4x128, 512] = Wx.T @ xT + Wh.T @ hT   (8 matmuls into one
        4-bank PSUM tile), ScalarE tanh/sigmoid with per-partition bias,
      VectorE: c = zf*c_ + zi*z ; h = zo*tanh(c),
      GEMM2: yT[128, 512] = w_out.T.T @ hT_rounded, ScalarE sigmoid + bias.
  - Outputs (c, h, y) come back as [128, B_shard] and are re-transposed and
    concatenated on the host.
"""

import numpy as np

B = 262144
F = 128          # feature dim (I = H = C = O = 128)
N_CORES = 8
B_SH = B // N_CORES          # 32768 rows per core
ST = 2048                    # supertile batch columns
N_ST = B_SH // ST            # 16 supertiles
WAVE = 512                   # wave batch columns
N_WAVES = ST // WAVE         # 4 waves per supertile

_PROGRAM_CACHE = {}


def _build_program():
    import concourse.mybir as mybir
    import concourse.tile as tile
    from concourse import bacc

    dt = mybir.dt
    Act = mybir.ActivationFunctionType

    nc = bacc.Bacc("TRN2", debug=False, num_devices=N_CORES)

    # big streamed tensors, [feature, batch_shard] layout
    xT = nc.declare_dram_parameter("xT", [F, B_SH], dt.float32r, isOutput=False)
    hT = nc.declare_dram_parameter("hT", [F, B_SH], dt.float32r, isOutput=False)
    cT = nc.declare_dram_parameter("cT", [F, B_SH], dt.float32, isOutput=False)
    cT_o = nc.declare_dram_parameter("cT_o", [F, B_SH], dt.float32, isOutput=True)
    hT_o = nc.declare_dram_parameter("hT_o", [F, B_SH], dt.float32, isOutput=True)
    yT_o = nc.declare_dram_parameter("yT_o", [F, B_SH], dt.float32, isOutput=True)

    # replicated weights (host-prepared layouts)
    wx = nc.declare_dram_parameter("wx", [F, 512], dt.float32r, isOutput=False)
    wh = nc.declare_dram_parameter("wh", [F, 512], dt.float32r, isOutput=False)
    wo = nc.declare_dram_parameter("wo", [F, F], dt.float32r, isOutput=False)
    bg = nc.declare_dram_parameter("bg", [F, 4], dt.float32, isOutput=False)
    bo2 = nc.declare_dram_parameter("bo2", [F, 1], dt.float32, isOutput=False)
    # sigmoid-gate biases as a K=1 matmul lhsT row [1, 3*F] (order i, f, o)
    bsig = nc.declare_dram_parameter("bsig", [1, 3 * F], dt.float32r, isOutput=False)
    ones = nc.declare_dram_parameter("ones", [1, WAVE], dt.float32r, isOutput=False)

    with tile.TileContext(nc) as tc:
        with (
            tc.tile_pool(name="wpool", bufs=1) as wpool,
            tc.tile_pool(name="io", bufs=3) as io,
            tc.tile_pool(name="oio", bufs=3) as oio,
            tc.tile_pool(name="tmps", bufs=3) as tmps,
            tc.tile_pool(name="ptmps", bufs=2) as ptmps,
            tc.tile_pool(name="gpsum", bufs=2, space="PSUM") as gpsum,
            tc.tile_pool(name="zypsum", bufs=2, space="PSUM") as zypsum,
        ):
            wx_sb = wpool.tile([F, 512], dt.float32r, tag="wx")
            wh_sb = wpool.tile([F, 512], dt.float32r, tag="wh")
            wo_sb = wpool.tile([F, F], dt.float32r, tag="wo")
            bg_sb = wpool.tile([F, 4], dt.float32, tag="bg")
            bo2_sb = wpool.tile([F, 1], dt.float32, tag="bo2")
            bsig_sb = wpool.tile([1, 3 * F], dt.float32r, tag="bsig")
            ones_sb = wpool.tile([1, WAVE], dt.float32r, tag="ones")
            nc.sync.dma_start(wx_sb[:], wx[:])
            nc.sync.dma_start(wh_sb[:], wh[:])
            nc.sync.dma_start(wo_sb[:], wo[:])
            nc.sync.dma_start(bg_sb[:], bg[:])
            nc.sync.dma_start(bo2_sb[:], bo2[:])
            nc.sync.dma_start(bsig_sb[:], bsig[:])
            nc.sync.dma_start(ones_sb[:], ones[:])

            for s in range(N_ST):
                ss = slice(s * ST, (s + 1) * ST)
                xr = io.tile([F, ST], dt.float32r, tag="xr")
                hr = io.tile([F, ST], dt.float32r, tag="hr")
                ci = io.tile([F, ST], dt.float32, tag="ci")
                nc.sync.dma_start(xr[:], xT[:, ss])
                nc.sync.dma_start(hr[:], hT[:, ss])
                nc.sync.dma_start(ci[:], cT[:, ss])

                cto = oio.tile([F, ST], dt.float32, tag="cto")
                hto = oio.tile([F, ST], dt.float32, tag="hto")
                yto = oio.tile([F, ST], dt.float32, tag="yto")

                for wv in range(N_WAVES):
                    bs = slice(wv * WAVE, (wv + 1) * WAVE)
                    # z gate: 1-bank PSUM from the shared z/y pool; bias fused
                    # into its tanh. Sigmoid gates: one 3-bank tile, biases
                    # injected by K=1 matmuls so ONE big sigmoid op drains it.
                    zp = zypsum.tile([F, WAVE], dt.float32, tag="zy")
                    nc.tensor.matmul(zp[:], wx_sb[:, 0:F], xr[:, bs],
                                     start=True, stop=False)
                    nc.tensor.matmul(zp[:], wh_sb[:, 0:F], hr[:, bs],
                                     start=False, stop=True)
                    gs = gpsum.tile([F, 3, WAVE], dt.float32, tag="gs")
                    for j in range(3):
                        gsl = slice((j + 1) * F, (j + 2) * F)
                        nc.tensor.matmul(gs[:, j, :],
                                         bsig_sb[:, j * F:(j + 1) * F],
                                         ones_sb[:], start=True, stop=False)
                        nc.tensor.matmul(gs[:, j, :], wx_sb[:, gsl], xr[:, bs],
                                         start=False, stop=False)
                        nc.tensor.matmul(gs[:, j, :], wh_sb[:, gsl], hr[:, bs],
                                         start=False, stop=True)

                    z = tmps.tile([F, WAVE], dt.float32, tag="z")
                    sg = tmps.tile([F, 3, WAVE], dt.float32, tag="sg")
                    nc.scalar.activation(z[:], zp[:], Act.Tanh, bias=bg_sb[:, 0:1])
                    nc.scalar.activation(sg[:], gs[:], Act.Sigmoid)
                    zi, zf, zo = sg[:, 0, :], sg[:, 1, :], sg[:, 2, :]

                    tmp = tmps.tile([F, WAVE], dt.float32, tag="tmp")
                    nc.vector.tensor_mul(tmp[:], zi, z[:])
                    nc.vector.tensor_mul(cto[:, bs], zf, ci[:, bs])
                    nc.vector.tensor_add(cto[:, bs], cto[:, bs], tmp[:])

                    if wv % 2 == 1:
                        # pair tanh(c) + the f32r cast across two waves to
                        # amortize per-op overhead
                        ps = slice((wv - 1) * WAVE, (wv + 1) * WAVE)
                        b0 = slice((wv - 1) * WAVE, wv * WAVE)
                        tw = ptmps.tile([F, 2 * WAVE], dt.float32, tag="tw")
                        nc.scalar.activation(tw[:], cto[:, ps], Act.Tanh)
                        nc.vector.tensor_mul(hto[:, b0], zo_prev, tw[:, :WAVE])
                        nc.vector.tensor_mul(hto[:, bs], zo, tw[:, WAVE:])

                        hr2 = ptmps.tile([F, 2 * WAVE], dt.float32r, tag="hr2")
                        nc.vector.tensor_copy(hr2[:], hto[:, ps])
                        for k, bk in ((0, b0), (1, bs)):
                            yp = zypsum.tile([F, WAVE], dt.float32, tag="zy")
                            nc.tensor.matmul(yp[:], wo_sb[:],
                                             hr2[:, k * WAVE:(k + 1) * WAVE],
                                             start=True, stop=True)
                            nc.scalar.activation(yto[:, bk], yp[:],
                                                 Act.Sigmoid, bias=bo2_sb[:])
                    else:
                        zo_prev = zo

                nc.sync.dma_start(cT_o[:, ss], cto[:])
                nc.sync.dma_start(hT_o[:, ss], hto[:])
                nc.sync.dma_start(yT_o[:, ss], yto[:])

    nc.finalize()
    return nc


def kernel(c_, h_, x, w, wi, wf, wo, w_out, b, bi, bf, bo, b_out):
    from concourse.bass_utils import run_bass_kernel_spmd

    if "nc" not in _PROGRAM_CACHE:
        _PROGRAM_CACHE["nc"] = _build_program()
    nc = _PROGRAM_CACHE["nc"]

    c_ = np.asarray(c_, dtype=np.float32)
    h_ = np.asarray(h_, dtype=np.float32)
    x = np.asarray(x, dtype=np.float32)

    # host weight prep: W_stack rows ordered [z, i, f, o]
    W_stack = np.concatenate(
        [np.asarray(a, np.float32) for a in (w, wi, wf, wo)], axis=0
    )  # [512, 256]
    wx_h = np.ascontiguousarray(W_stack[:, :F].T)    # [128, 512]
    wh_h = np.ascontiguousarray(W_stack[:, F:].T)    # [128, 512]
    wo_h = np.ascontiguousarray(np.asarray(w_out, np.float32).T)  # [128, 128]
    bg_h = np.ascontiguousarray(
        np.stack(
            [np.asarray(v, np.float32) for v in (b, bi, bf, bo)], axis=1
        )
    )  # [128, 4]
    bo2_h = np.ascontiguousarray(np.asarray(b_out, np.float32).reshape(F, 1))
    bsig_h = np.ascontiguousarray(
        np.concatenate(
            [np.asarray(v, np.float32) for v in (bi, bf, bo)]
        ).reshape(1, 3 * F)
    )
    ones_h = np.ones((1, WAVE), np.float32)

    # shard along batch and transpose to [feature, batch]
    xs = x.reshape(N_CORES, B_SH, F)
    hs = h_.reshape(N_CORES, B_SH, F)
    cs = c_.reshape(N_CORES, B_SH, F)
    in_maps = []
    for i in range(N_CORES):
        in_maps.append(
            {
                "xT": np.ascontiguousarray(xs[i].T),
                "hT": np.ascontiguousarray(hs[i].T),
                "cT": np.ascontiguousarray(cs[i].T),
                "wx": wx_h,
                "wh": wh_h,
                "wo": wo_h,
                "bg": bg_h,
                "bo2": bo2_h,
                "bsig": bsig_h,
                "ones": ones_h,
            }
        )

    _PROGRAM_CACHE["in_maps"] = in_maps
    res = run_bass_kernel_spmd(nc, in_maps, list(range(N_CORES)))

    c_out = np.empty((B, F), np.float32)
    h_out = np.empty((B, F), np.float32)
    y_out = np.empty((B, F), np.float32)
    for i in range(N_CORES):
        r = res.results[i]
        sl = slice(i * B_SH, (i + 1) * B_SH)
        c_out[sl] = r["cT_o"].T
        h_out[sl] = r["hT_o"].T
        y_out[sl] = r["yT_o"].T
    return (c_out, h_out, y_out)


# revision 21
# speedup vs baseline: 1.2697x; 1.2697x over previous
"""Trainium2 Bass kernel for the LSTM-unit problem (B=262144, I=H=C=O=128).

Strategy (data-parallel over 8 NeuronCores, batch-sharded):
  - Host pre-transposes the big activations to [feature, batch] layout so the
    on-chip pipeline needs zero transposes: the TensorE contraction dim
    (partitions) is the feature dim for both GEMMs, gate biases become
    per-partition (fused into ScalarE activations for free), and the second
    GEMM consumes the h tiles the first stage produced.
  - GEMM operands use float32r (reduced-precision fp32 matmul mode): full
    1 elem/cycle PE rate at N=512 with ~1.6e-4 relative error.
  - Per core: stream 16 supertiles of 2048 batch rows; each supertile is
    4 waves of 512 batch columns:
      gates.T[4x128, 512] = Wx.T @ xT + Wh.T @ hT   (8 matmuls into one
        4-bank PSUM tile), ScalarE tanh/sigmoid with per-partition bias,
      VectorE: c = zf*c_ + zi*z ; h = zo*tanh(c),
      GEMM2: yT[128, 512] = w_out.T.T @ hT_rounded, ScalarE sigmoid + bias.
  - Outputs (c, h, y) come back as [128, B_shard] and are re-transposed and
    concatenated on the host.
"""

import numpy as np

B = 262144
F = 128          # feature dim (I = H = C = O = 128)
N_CORES = 8
B_SH = B // N_CORES          # 32768 rows per core
ST = 2048                    # supertile batch columns
N_ST = B_SH // ST            # 16 supertiles
WAVE = 512                   # wave batch columns
N_WAVES = ST // WAVE         # 4 waves per supertile

_PROGRAM_CACHE = {}


def _build_program():
    import concourse.mybir as mybir
    import concourse.tile as tile
    from concourse import bacc

    dt = mybir.dt
    Act = mybir.ActivationFunctionType

    nc = bacc.Bacc("TRN2", debug=False, num_devices=N_CORES)

    # big streamed tensors, [feature, batch_shard] layout
    xT = nc.declare_dram_parameter("xT", [F, B_SH], dt.float32r, isOutput=False)
    hT = nc.declare_dram_parameter("hT", [F, B_SH], dt.float32r, isOutput=False)
    cT = nc.declare_dram_parameter("cT", [F, B_SH], dt.float32, isOutput=False)
    cT_o = nc.declare_dram_parameter("cT_o", [F, B_SH], dt.float32, isOutput=True)
    hT_o = nc.declare_dram_parameter("hT_o", [F, B_SH], dt.float32, isOutput=True)
    yT_o = nc.declare_dram_parameter("yT_o", [F, B_SH], dt.float32, isOutput=True)

    # replicated weights (host-prepared layouts)
    wx = nc.declare_dram_parameter("wx", [F, 512], dt.float32r, isOutput=False)
    wh = nc.declare_dram_parameter("wh", [F, 512], dt.float32r, isOutput=False)
    wo = nc.declare_dram_parameter("wo", [F, F], dt.float32r, isOutput=False)
    bg = nc.declare_dram_parameter("bg", [F, 4], dt.float32, isOutput=False)
    bo2 = nc.declare_dram_parameter("bo2", [F, 1], dt.float32, isOutput=False)


    with tile.TileContext(nc) as tc:
        with (
            tc.tile_pool(name="wpool", bufs=1) as wpool,
            tc.tile_pool(name="io", bufs=3) as io,
            tc.tile_pool(name="oio", bufs=3) as oio,
            tc.tile_pool(name="tmps", bufs=3) as tmps,
            tc.tile_pool(name="ptmps", bufs=2) as ptmps,
            tc.tile_pool(name="gpsum", bufs=3, space="PSUM") as gpsum,
            tc.tile_pool(name="zypsum", bufs=1, space="PSUM") as zypsum,
        ):
            wx_sb = wpool.tile([F, 512], dt.float32r, tag="wx")
            wh_sb = wpool.tile([F, 512], dt.float32r, tag="wh")
            wo_sb = wpool.tile([F, F], dt.float32r, tag="wo")
            bg_sb = wpool.tile([F, 4], dt.float32, tag="bg")
            bo2_sb = wpool.tile([F, 1], dt.float32, tag="bo2")
            nc.sync.dma_start(wx_sb[:], wx[:])
            nc.sync.dma_start(wh_sb[:], wh[:])
            nc.sync.dma_start(wo_sb[:], wo[:])
            nc.sync.dma_start(bg_sb[:], bg[:])
            nc.sync.dma_start(bo2_sb[:], bo2[:])

            for s in range(N_ST):
                ss = slice(s * ST, (s + 1) * ST)
                xr = io.tile([F, ST], dt.float32r, tag="xr")
                hr = io.tile([F, ST], dt.float32r, tag="hr")
                ci = io.tile([F, ST], dt.float32, tag="ci")
                nc.sync.dma_start(xr[:], xT[:, ss])
                nc.sync.dma_start(hr[:], hT[:, ss])
                nc.sync.dma_start(ci[:], cT[:, ss])

                cto = oio.tile([F, ST], dt.float32, tag="cto")
                hto = oio.tile([F, ST], dt.float32, tag="hto")
                yto = oio.tile([F, ST], dt.float32, tag="yto")

                for wv in range(N_WAVES):
                    bs = slice(wv * WAVE, (wv + 1) * WAVE)
                    # two 2-bank PSUM tiles so the next wave's GEMM1 can start
                    # as soon as one pair of gates is drained by ScalarE
                    ga = gpsum.tile([F, 2, WAVE], dt.float32, tag="g")
                    gb = gpsum.tile([F, 2, WAVE], dt.float32, tag="g")
                    for gi in range(4):
                        gt = ga if gi < 2 else gb
                        gsl = slice(gi * F, (gi + 1) * F)
                        nc.tensor.matmul(
                            gt[:, gi % 2, :], wx_sb[:, gsl], xr[:, bs],
                            start=True, stop=False,
                        )
                        nc.tensor.matmul(
                            gt[:, gi % 2, :], wh_sb[:, gsl], hr[:, bs],
                            start=False, stop=True,
                        )

                    z = tmps.tile([F, WAVE], dt.float32, tag="z")
                    zi = tmps.tile([F, WAVE], dt.float32, tag="zi")
                    zf = tmps.tile([F, WAVE], dt.float32, tag="zf")
                    zo = tmps.tile([F, WAVE], dt.float32, tag="zo")
                    nc.scalar.activation(z[:], ga[:, 0, :], Act.Tanh, bias=bg_sb[:, 0:1])
                    nc.scalar.activation(zi[:], ga[:, 1, :], Act.Sigmoid, bias=bg_sb[:, 1:2])
                    nc.scalar.activation(zf[:], gb[:, 0, :], Act.Sigmoid, bias=bg_sb[:, 2:3])
                    nc.scalar.activation(zo[:], gb[:, 1, :], Act.Sigmoid, bias=bg_sb[:, 3:4])

                    tmp = tmps.tile([F, WAVE], dt.float32, tag="tmp")
                    nc.vector.tensor_mul(tmp[:], zi[:], z[:])
                    nc.vector.tensor_mul(cto[:, bs], zf[:], ci[:, bs])
                    nc.vector.tensor_add(cto[:, bs], cto[:, bs], tmp[:])

                    if wv % 2 == 1:
                        # pair the tail ops across two waves: one big tanh(c),
                        # one big hr cast, one big y-sigmoid — amortizes the
                        # ~352-cycle ScalarE per-op overhead
                        ps = slice((wv - 1) * WAVE, (wv + 1) * WAVE)
                        b0 = slice((wv - 1) * WAVE, wv * WAVE)
                        tw = ptmps.tile([F, 2 * WAVE], dt.float32, tag="tw")
                        nc.scalar.activation(tw[:], cto[:, ps], Act.Tanh)
                        nc.vector.tensor_mul(hto[:, b0], zo_prev[:], tw[:, :WAVE])
                        nc.vector.tensor_mul(hto[:, bs], zo[:], tw[:, WAVE:])

                        hr2 = ptmps.tile([F, 2 * WAVE], dt.float32r, tag="hr2")
                        nc.vector.tensor_copy(hr2[:], hto[:, ps])
                        yp = zypsum.tile([F, 2, WAVE], dt.float32, tag="yp")
                        nc.tensor.matmul(yp[:, 0, :], wo_sb[:], hr2[:, :WAVE],
                                         start=True, stop=True)
                        nc.tensor.matmul(yp[:, 1, :], wo_sb[:], hr2[:, WAVE:],
                                         start=True, stop=True)
                        nc.scalar.activation(yto[:, ps], yp[:, :, :],
                                             Act.Sigmoid, bias=bo2_sb[:])
                    else:
                        zo_prev = zo

                nc.sync.dma_start(cT_o[:, ss], cto[:])
                nc.sync.dma_start(hT_o[:, ss], hto[:])
                nc.sync.dma_start(yT_o[:, ss], yto[:])

    nc.finalize()
    return nc


def kernel(c_, h_, x, w, wi, wf, wo, w_out, b, bi, bf, bo, b_out):
    from concourse.bass_utils import run_bass_kernel_spmd

    if "nc" not in _PROGRAM_CACHE:
        _PROGRAM_CACHE["nc"] = _build_program()
    nc = _PROGRAM_CACHE["nc"]

    c_ = np.asarray(c_, dtype=np.float32)
    h_ = np.asarray(h_, dtype=np.float32)
    x = np.asarray(x, dtype=np.float32)

    # host weight prep: W_stack rows ordered [z, i, f, o]
    W_stack = np.concatenate(
        [np.asarray(a, np.float32) for a in (w, wi, wf, wo)], axis=0
    )  # [512, 256]
    wx_h = np.ascontiguousarray(W_stack[:, :F].T)    # [128, 512]
    wh_h = np.ascontiguousarray(W_stack[:, F:].T)    # [128, 512]
    wo_h = np.ascontiguousarray(np.asarray(w_out, np.float32).T)  # [128, 128]
    bg_h = np.ascontiguousarray(
        np.stack(
            [np.asarray(v, np.float32) for v in (b, bi, bf, bo)], axis=1
        )
    )  # [128, 4]
    bo2_h = np.ascontiguousarray(np.asarray(b_out, np.float32).reshape(F, 1))

    # shard along batch and transpose to [feature, batch]
    xs = x.reshape(N_CORES, B_SH, F)
    hs = h_.reshape(N_CORES, B_SH, F)
    cs = c_.reshape(N_CORES, B_SH, F)
    in_maps = []
    for i in range(N_CORES):
        in_maps.append(
            {
                "xT": np.ascontiguousarray(xs[i].T),
                "hT": np.ascontiguousarray(hs[i].T),
                "cT": np.ascontiguousarray(cs[i].T),
                "wx": wx_h,
                "wh": wh_h,
                "wo": wo_h,
                "bg": bg_h,
                "bo2": bo2_h,
            }
        )

    _PROGRAM_CACHE["in_maps"] = in_maps
    res = run_bass_kernel_spmd(nc, in_maps, list(range(N_CORES)))

    c_out = np.empty((B, F), np.float32)
    h_out = np.empty((B, F), np.float32)
    y_out = np.empty((B, F), np.float32)
    for i in range(N_CORES):
        r = res.results[i]
        sl = slice(i * B_SH, (i + 1) * B_SH)
        c_out[sl] = r["cT_o"].T
        h_out[sl] = r["hT_o"].T
        y_out[sl] = r["yT_o"].T
    return (c_out, h_out, y_out)
